# revision 32
# baseline (speedup 1.0000x reference)
"""Trainium2 Bass kernel for nn_Encoder_23124103922122 (segment_reduce), v5.

Math (per rank r of 6, labels lab_r[0..4095] in [0,256)):
    seg_r[b, g]  = sum_{i: lab_r[i]==g} F[b, i]          (segment sum)
    out[b, j, r] = seg_r[b, lab_r[j]]                     (gather back)
    out[b, j, 6] = F[b, j]                                (identity channel)

v6 design — channel-major output, rank-pipelined, fp8 DoubleRow stage-2
(74.3us -> 47.1us vs the v2 interleaved design):
  - The device computes only the 6 rank channels, laid out channel-major:
    out2[b, r, j] (bf16). The host transposes to [b, j, r] and fills the
    identity channel directly from the f32 input (exact). This removes v2's
    all-ranks interleave barrier: rank r's channel streams to HBM (in
    quarters) as soon as its stage-2 strips finish, and the device output
    shrinks from 14.7MB to 6.3MB per core.
  - Stage-1 per rank: mask-as-STATIONARY (psum_segT[gh, b] += m1[i, gh].T
    @ f_t[i, b]), 64 bf16 matmuls (gh-outer: interleaved accumulation
    groups miscompute on HW). m1 masks (iota_g u16 vs f32 label scalar,
    bf16 out, DVE 4x mode) 21/rank on DVE + 11/rank on Pool.
  - Stage-2 per rank: seg is split into fp8e4 hi (seg8) + fp8 residual
    (res8) — together ~2^-8 accurate — and gathered with fp8 DoubleRow
    matmuls (2 k-tiles per instruction, 0.5 cyc/row: 4x bf16 throughput).
    The one-hot m2 masks arrive pre-packed [g, ktile, j] in fp8 from the
    host (6MB DMA, replacing v5's on-device DVE mask builds byte-for-byte
    against the label broadcast they needed).
  - Software-pipelined two ranks deep at the ramp (s1(0) s1(1) s2(0) s1(2)
    ...), so stage-2 deps (m28 DMA, fp8 seg prep) never stall PE; the final
    two ranks' strips interleave 6/6-2/2 so only ~0.5MB streams after the
    last matmul. po->channel copies pair two 256-strips per op, mostly ACT
    (DVE takes the tail ranks). PE ~33us busy, DMA ~38us busy of 47us.

Sharding: data-parallel over batch B=1024 -> 8 cores x 128 rows. Labels
replicated. No cross-device communication.

Note: walrus in this container accepts at most ONE sync-wait per instruction
(two on EventSemaphore); _legalize_waits() post-processes the Tile-scheduled
program to satisfy that.
"""

import sys

if "/opt/trn_rl_repo" not in sys.path:
    sys.path.insert(0, "/opt/trn_rl_repo")

from contextlib import ExitStack

import ml_dtypes
import numpy as np

import concourse.bass as bass
import concourse.mybir as mybir
import concourse.tile as tile
from concourse.bass import ts
from concourse.bass_utils import run_bass_kernel_spmd

B, N, R, G = 1024, 4096, 6, 256
NCORES = 8
BL = B // NCORES  # 128 batch rows per core
P = 128
NT = N // P  # 32 genus tiles
F32 = mybir.dt.float32
BF16 = mybir.dt.bfloat16
U16 = mybir.dt.uint16
FP8 = mybir.dt.float8e4

# m1 mask tiles built on Pool (the rest go to DVE). Pool is ~4.2x slower
# per tile, so its tiles sit late in the rank (PE reaches them last) and the
# ramp ranks (whose windows are half-length) get fewer of them.
POOL_T_RAMP = {13, 19, 25, 31}
POOL_T = {9, 12, 14, 16, 18, 20, 22, 24, 26, 29, 31}
_cache: dict = {}

# Engine -> prefix of the semaphore names its compute instructions increment.
# Pool (GPSIMD) is excluded: its 8 DSP cores do not guarantee in-order
# completion, so Pool-on-Pool waits cannot be dropped as redundant.
_ENGINE_SEM_PREFIX = {
    mybir.EngineType.PE: "PE",
    mybir.EngineType.DVE: "DVE",
    mybir.EngineType.Activation: "Activation",
    mybir.EngineType.SP: "SP",
}


def _legalize_waits(nc):
    """Walrus only accepts 1 sync-wait per instruction (2 on EventSemaphore),
    but the Tile scheduler can emit more. Post-pass:
      1. drop waits on the instruction's own engine semaphore that are already
         satisfied by same-engine program order (compute completion is in-order
         and sem targets are absolute), and
      2. hoist remaining excess waits onto EventSemaphore carrier instructions
         inserted just before the instruction on the same engine.
    """
    ev_id = 0
    for f in nc.m.functions:
        for blk in f.blocks:
            insts = blk.instructions
            sem_incs: dict = {}  # (engine, sem_name) -> cumulative inc in stream
            new_insts = []
            for inst in insts:
                si = inst.sync_info
                if si is not None and si.on_wait:
                    cap = 2 if isinstance(inst, mybir.InstEventSemaphore) else 1
                    eng = inst.engine
                    pfx = _ENGINE_SEM_PREFIX.get(eng)
                    kept = []
                    for w in si.on_wait:
                        sem_eng = w.ant_name.rsplit("_", 1)[0]
                        if (
                            pfx is not None
                            and sem_eng == pfx
                            and w.wait_mode == "sem-ge-imm"
                            and sem_incs.get((eng, w.ant_name), 0) >= w.wait_value
                        ):
                            continue  # satisfied by same-engine execution order
                        kept.append(w)
                    while len(kept) > cap:
                        ncarry = min(2, len(kept) - cap + 1)
                        carry, kept = kept[:ncarry], kept[ncarry:]
                        ev = mybir.InstEventSemaphore(
                            name=f"EVW-{ev_id}", ins=[], outs=[]
                        )
                        ev_id += 1
                        ev.engine = eng
                        ev.sync_info = mybir.SyncInfo(on_wait=carry, on_update=[])
                        new_insts.append(ev)
                    inst.sync_info = mybir.SyncInfo(
                        on_wait=kept, on_update=si.on_update
                    )
                si = inst.sync_info
                if si is not None:
                    for u in si.on_update:
                        if u.update_mode == "sem-inc":
                            key = (inst.engine, u.ant_name)
                            sem_incs[key] = sem_incs.get(key, 0) + u.update_value
                new_insts.append(inst)
            if len(new_insts) != len(insts):
                insts[:] = new_insts


def _build_nc():
    nc = bass.Bass("TRN2", debug=False, num_devices=NCORES)

    # f_t[p, t*128 + b] = F[b, t*128 + p]  (bf16 transposed F tiles)
    f_t_in = nc.dram_tensor("f_t_in", [P, N], BF16, kind="ExternalInput").ap()
    # tabs16[p, 0:G] = iota_g; [G:G+2] = (p, p+128); [G+2+r*NT+t] =
    # labels[r, t*128+p] -- one u16 table tensor, one DMA
    tabs16 = nc.dram_tensor(
        "tabs16", [P, G + 2 + R * NT], U16, kind="ExternalInput"
    ).ap()
    # m28[r, g, h, j] = (labels[r, j] == h*128 + g), fp8e4 (host-built
    # one-hot gather masks, DoubleRow-packed: h is the k-tile axis)
    m28_in = nc.dram_tensor("m28", [R, P, 2, N], FP8, kind="ExternalInput").ap()
    # channel-major output: out2[b, r, j] = seg_r[b, lab_r[j]]
    out2 = nc.dram_tensor("out2", [BL, R, N], BF16, kind="ExternalOutput").ap()

    with ExitStack() as ctx:
        tc = ctx.enter_context(tile.TileContext(nc))

        const = ctx.enter_context(tc.tile_pool(name="const", bufs=1))
        m1p = ctx.enter_context(tc.tile_pool(name="m1p", bufs=52))
        m1pp = ctx.enter_context(tc.tile_pool(name="m1pp", bufs=22))
        segp = ctx.enter_context(tc.tile_pool(name="segp", bufs=3))
        chp = ctx.enter_context(tc.tile_pool(name="chp", bufs=3))
        ps_seg = ctx.enter_context(tc.tile_pool(name="ps_seg", bufs=2, space="PSUM"))
        ps_o = ctx.enter_context(tc.tile_pool(name="ps_o", bufs=5, space="PSUM"))

        # ---- input DMAs. sync (SP) queue: tables, then f_t (compute-
        # critical, in quarters so stage-1 starts early), then the 6 per-rank
        # u16 label broadcasts (rank r's is needed only by its m2 build). ----
        t16_sb = const.tile([P, G + 2 + R * NT], U16)
        nc.sync.dma_start(t16_sb[:], tabs16)
        f_t = const.tile([P, N], BF16)
        j0 = 0
        for cw in (256, 384, 512, 640, 768, 896, 640):
            nc.sync.dma_start(f_t[:, j0 : j0 + cw], f_t_in[:, j0 : j0 + cw])
            j0 += cw
        m2_sb = const.tile([P, R, 2, N], FP8)
        for r in range(R):
            nc.sync.dma_start(m2_sb[:, r], m28_in[r])

        iota_g_sb = t16_sb[:, 0:G]
        # is_equal scalars must be f32: convert the iota_p/labT columns once
        scal_sb = const.tile([P, 2 + R * NT], F32)
        nc.scalar.copy(scal_sb[:], t16_sb[:, G:])
        labT_sb = scal_sb[:, 2:]

        # ---- prewarm: absorb const-DMA semaphores into the DVE/Pool clocks
        # (hot-loop ops may carry at most one sync wait), and keep PE busy on
        # junk matmuls until f_t arrives so its p-state ramps. ----
        warm = const.tile([P, 4], BF16)
        wjunk = const.tile([P, P], BF16)
        nc.vector.memset(wjunk[:], 0.0)
        nc.vector.tensor_copy(warm[:, 0:1], t16_sb[:, 0:1])
        nc.gpsimd.tensor_copy(warm[:, 2:3], t16_sb[:, 1:2])
        with tc.tile_pool(name="ps_warm", bufs=1, space="PSUM") as ps_warm:
            wps = ps_warm.tile([P, P], F32)
            # p-state warm-up on UNINITIALIZED tiles: no input dependency, so
            # PE is busy from ~0.6us (the product is never read)
            for _ in range(28):
                nc.tensor.matmul(wps[:], wjunk[:], wjunk[:], start=True, stop=True)

        # ---- per-rank emitters ----
        seg_ps = {}
        seg_t = {}

        def emit_m1(r):
            masks = []
            pool_t = POOL_T_RAMP if r < 2 else POOL_T
            for t in range(NT):
                col = r * NT + t
                if t in pool_t:
                    mt = m1pp.tile([P, G], BF16, tag="m1p", name=f"m1p{r}_{t}")
                    eng = nc.gpsimd
                else:
                    mt = m1p.tile([P, G], BF16, tag="m1", name=f"m1_{r}_{t}")
                    eng = nc.vector
                eng.tensor_scalar(
                    mt[:],
                    iota_g_sb[:],
                    labT_sb[:, col : col + 1],
                    None,
                    op0=mybir.AluOpType.is_equal,
                )
                masks.append(mt)
            return masks

        def emit_s1(r, masks):
            # gh outer / t inner: interleaved accumulation groups miscompute
            # on HW (probe-verified), so groups must stay contiguous
            t_ = ps_seg.tile([P, 2, P], F32, tag="segps", name=f"segps{r}")
            seg_ps[r] = t_
            for gh in range(2):
                for t in range(NT):
                    nc.tensor.matmul(
                        t_[:, gh, :],
                        masks[t][:, ts(gh, P)],
                        f_t[:, ts(t, P)],
                        start=(t == 0),
                        stop=(t == NT - 1),
                    )

        def emit_seg_copy(r):
            # seg -> fp8 hi (seg8) + fp8 residual (res8); seg8 + res8
            # reconstructs seg to ~2^-8 relative. All ops same-dtype pairs.
            sbf = segp.tile([P, 2, P], BF16, tag="sbf", name=f"sbf{r}")
            nc.vector.tensor_copy(sbf[:], seg_ps[r][:])
            s8 = segp.tile([P, 2, P], FP8, tag="s8", name=f"s8_{r}")
            nc.scalar.copy(s8[:], seg_ps[r][:])
            s8b = segp.tile([P, 2, P], BF16, tag="s8b", name=f"s8b{r}")
            nc.scalar.copy(s8b[:], s8[:])
            rbf = segp.tile([P, 2, P], BF16, tag="rbf", name=f"rbf{r}")
            nc.vector.tensor_tensor(
                rbf[:], sbf[:], s8b[:], op=mybir.AluOpType.subtract
            )
            r8 = segp.tile([P, 2, P], FP8, tag="r8", name=f"r8_{r}")
            nc.scalar.copy(r8[:], rbf[:])
            seg_t[r] = (s8, r8)

        ch_of = {}
        W2 = 256  # DoubleRow strip width (rhs free = 2*W2 = PE moving max)

        def emit_s2(r, p_lo=0, p_hi=8):
            # stage-2: per 256-strip, two fp8 DoubleRow matmuls (seg8 then
            # res8) contract all 256 groups at 0.5 cyc/row. Strips pair up in
            # one psum tile so each po->channel copy moves 512 columns.
            if r in ch_of:
                ch = ch_of[r]
            else:
                ch = chp.tile([P, N], BF16, tag="ch", name=f"ch{r}")
                ch_of[r] = ch
            s8, r8 = seg_t[r]
            for p_ in range(p_lo, p_hi):
                po = ps_o.tile([P, 2, W2], F32, tag="po", name=f"po{r}_{p_}")
                for half in range(2):
                    s = 2 * p_ + half
                    for op8 in (s8, r8):
                        nc.tensor.matmul(
                            po[:, half, :],
                            op8[:],
                            m2_sb[:, r, :, ts(s, W2)],
                            start=(op8 is s8),
                            stop=(op8 is r8),
                            perf_mode=mybir.MatmulPerfMode.DoubleRow,
                        )
                dve = p_ % 2 == 1 if r >= R - 2 else p_ % 4 == 2
                if dve:
                    nc.vector.tensor_copy(ch[:, ts(p_, 2 * W2)], po[:])
                else:
                    nc.scalar.copy(ch[:, ts(p_, 2 * W2)], po[:])
                # stream in quarters (2 pairs each): short final drain,
                # cheap per-DMA HWDGE hold (625ns, exclusive)
                if p_ % 2 == 1:
                    q = p_ // 2
                    nc.sync.dma_start(
                        out2[:, r, ts(q, N // 4)], ch[:, ts(q, N // 4)]
                    )

        # ---- software-pipelined emission: PE order is
        # s1(0) s1(1) s2(0) s1(2) s2(1) ... s1(5) s2(4) s2(5), so PE always
        # has a stage-1 queued while stage-2 deps (m2, seg copies) settle.
        # m2(r+1) is emitted during s2(r)'s window so DVE never idles on a
        # label DMA and each m2 is ready one full window early. ----
        masks0 = emit_m1(0)
        emit_s1(0, masks0)
        emit_seg_copy(0)
        masks1 = emit_m1(1)
        emit_s1(1, masks1)
        emit_seg_copy(1)
        for r in range(4):
            emit_s2(r)
            masks = emit_m1(r + 2)
            emit_s1(r + 2, masks)
            emit_seg_copy(r + 2)
        # tail: interleave ranks 4/5 stage-2 halves so rank 5's seg prep
        # settles during rank 4's strips and the final drain is ~0.5MB
        emit_s2(4, 0, 6)
        emit_s2(5, 0, 6)
        emit_s2(4, 6, 8)
        emit_s2(5, 6, 8)

    _legalize_waits(nc)
    return nc


def _host_tables(labels):
    # tabs16 = [iota_g | iota_p, iota_p+128 | labT] as u16
    iota_g = np.tile(np.arange(G, dtype=np.uint16), (P, 1))
    iota_p = (
        np.arange(P, dtype=np.uint16)[:, None]
        + np.uint16(128) * np.arange(2, dtype=np.uint16)[None, :]
    )
    labT = (
        np.transpose(labels.reshape(R, NT, P), (2, 0, 1))
        .reshape(P, R * NT)
        .astype(np.uint16)
    )
    return np.ascontiguousarray(
        np.concatenate([iota_g, iota_p, labT], axis=1).astype(np.uint16)
    )


def kernel(F_genus: np.ndarray, labels: np.ndarray) -> np.ndarray:
    F_genus = np.ascontiguousarray(F_genus, dtype=np.float32)
    labels = np.ascontiguousarray(labels, dtype=np.int32)
    assert F_genus.shape == (B, N) and labels.shape == (R, N)

    tabs16 = _host_tables(labels)
    # m28[r, g, h, j] = (labels[r, j] == h*128 + g) in fp8e4 (exact 0/1)
    gvals = (
        np.arange(P, dtype=np.int32)[None, :, None, None]
        + 128 * np.arange(2, dtype=np.int32)[None, None, :, None]
    )
    m28 = np.ascontiguousarray(
        (labels[:, None, None, :] == gvals).astype(ml_dtypes.float8_e4m3fn)
    )

    in_maps = []
    for c in range(NCORES):
        Fc = F_genus[c * BL : (c + 1) * BL]  # [BL, N]
        # f_t[p, t*128 + b] = Fc[b, t*128 + p]
        f_t = np.ascontiguousarray(
            Fc.reshape(BL, NT, P).transpose(2, 1, 0).reshape(P, N)
        ).astype(ml_dtypes.bfloat16)
        in_maps.append(
            {
                "f_t_in": f_t,
                "tabs16": tabs16,
                "m28": m28,
            }
        )

    # The first execution of a freshly compiled NEFF occasionally hits a
    # transient NRT_EXEC_UNIT_UNRECOVERABLE; a rebuild + retry recovers.
    last_err = None
    for attempt in range(3):
        try:
            if "nc" not in _cache:
                _cache["nc"] = _build_nc()
            res = run_bass_kernel_spmd(
                _cache["nc"], in_maps, core_ids=list(range(NCORES))
            )
            out = np.empty((B, N, R + 1), dtype=np.float32)
            for c in range(NCORES):
                # out2 is [BL, R, N] bf16, channel-major -> transpose
                out[c * BL : (c + 1) * BL, :, :R] = (
                    res.results[c]["out2"].astype(np.float32).transpose(0, 2, 1)
                )
            out[:, :, R] = F_genus  # identity channel, exact
            return out
        except Exception as e:  # noqa: BLE001
            last_err = e
            _cache.pop("nc", None)
            import time as _time

            _time.sleep(3.0)
    raise last_err


# revision 43
# speedup vs baseline: 1.0319x; 1.0319x over previous
"""Trainium2 Bass kernel for nn_Encoder_23124103922122 (segment_reduce), v5.

Math (per rank r of 6, labels lab_r[0..4095] in [0,256)):
    seg_r[b, g]  = sum_{i: lab_r[i]==g} F[b, i]          (segment sum)
    out[b, j, r] = seg_r[b, lab_r[j]]                     (gather back)
    out[b, j, 6] = F[b, j]                                (identity channel)

v6 design — channel-major output, rank-pipelined, fp8 DoubleRow stage-2
(74.3us -> 47.1us vs the v2 interleaved design):
  - The device computes only the 6 rank channels, laid out channel-major:
    out2[b, r, j] (bf16). The host transposes to [b, j, r] and fills the
    identity channel directly from the f32 input (exact). This removes v2's
    all-ranks interleave barrier: rank r's channel streams to HBM (in
    quarters) as soon as its stage-2 strips finish, and the device output
    shrinks from 14.7MB to 6.3MB per core.
  - Stage-1 per rank: mask-as-STATIONARY (psum_segT[gh, b] += m1[i, gh].T
    @ f_t[i, b]), 64 bf16 matmuls (gh-outer: interleaved accumulation
    groups miscompute on HW). m1 masks (iota_g u16 vs f32 label scalar,
    bf16 out, DVE 4x mode) 21/rank on DVE + 11/rank on Pool.
  - Stage-2 per rank: seg is split into fp8e4 hi (seg8) + fp8 residual
    (res8) — together ~2^-8 accurate — and gathered with fp8 DoubleRow
    matmuls (2 k-tiles per instruction, 0.5 cyc/row: 4x bf16 throughput).
    The one-hot m2 masks arrive pre-packed [g, ktile, j] in fp8 from the
    host (6MB DMA, replacing v5's on-device DVE mask builds byte-for-byte
    against the label broadcast they needed).
  - Software-pipelined two ranks deep at the ramp (s1(0) s1(1) s2(0) s1(2)
    ...), so stage-2 deps (m28 DMA, fp8 seg prep) never stall PE; the final
    two ranks' strips interleave 6/6-2/2 so only ~0.5MB streams after the
    last matmul. po->channel copies pair two 256-strips per op, mostly ACT
    (DVE takes the tail ranks). PE ~33us busy, DMA ~38us busy of 47us.

Sharding: data-parallel over batch B=1024 -> 8 cores x 128 rows. Labels
replicated. No cross-device communication.

Note: walrus in this container accepts at most ONE sync-wait per instruction
(two on EventSemaphore); _legalize_waits() post-processes the Tile-scheduled
program to satisfy that.
"""

import sys

if "/opt/trn_rl_repo" not in sys.path:
    sys.path.insert(0, "/opt/trn_rl_repo")

from contextlib import ExitStack

import ml_dtypes
import numpy as np

import concourse.bass as bass
import concourse.mybir as mybir
import concourse.tile as tile
from concourse.bass import ts
from concourse.bass_utils import run_bass_kernel_spmd

B, N, R, G = 1024, 4096, 6, 256
NCORES = 8
BL = B // NCORES  # 128 batch rows per core
P = 128
NT = N // P  # 32 genus tiles
F32 = mybir.dt.float32
BF16 = mybir.dt.bfloat16
U16 = mybir.dt.uint16
FP8 = mybir.dt.float8e4

# m1 mask tiles built on Pool (the rest go to DVE). Pool is ~4.2x slower
# per tile, so its tiles sit late in the rank (PE reaches them last) and the
# ramp ranks (whose windows are half-length) get fewer of them.
POOL_T_RAMP = {11, 15, 19, 23, 27, 31}
POOL_T = {9, 12, 14, 16, 18, 20, 22, 24, 26, 29, 31}
_cache: dict = {}

# Engine -> prefix of the semaphore names its compute instructions increment.
# Pool (GPSIMD) is excluded: its 8 DSP cores do not guarantee in-order
# completion, so Pool-on-Pool waits cannot be dropped as redundant.
_ENGINE_SEM_PREFIX = {
    mybir.EngineType.PE: "PE",
    mybir.EngineType.DVE: "DVE",
    mybir.EngineType.Activation: "Activation",
    mybir.EngineType.SP: "SP",
}


def _legalize_waits(nc):
    """Walrus only accepts 1 sync-wait per instruction (2 on EventSemaphore),
    but the Tile scheduler can emit more. Post-pass:
      1. drop waits on the instruction's own engine semaphore that are already
         satisfied by same-engine program order (compute completion is in-order
         and sem targets are absolute), and
      2. hoist remaining excess waits onto EventSemaphore carrier instructions
         inserted just before the instruction on the same engine.
    """
    ev_id = 0
    for f in nc.m.functions:
        for blk in f.blocks:
            insts = blk.instructions
            sem_incs: dict = {}  # (engine, sem_name) -> cumulative inc in stream
            new_insts = []
            for inst in insts:
                si = inst.sync_info
                if si is not None and si.on_wait:
                    cap = 2 if isinstance(inst, mybir.InstEventSemaphore) else 1
                    eng = inst.engine
                    pfx = _ENGINE_SEM_PREFIX.get(eng)
                    kept = []
                    for w in si.on_wait:
                        sem_eng = w.ant_name.rsplit("_", 1)[0]
                        if (
                            pfx is not None
                            and sem_eng == pfx
                            and w.wait_mode == "sem-ge-imm"
                            and sem_incs.get((eng, w.ant_name), 0) >= w.wait_value
                        ):
                            continue  # satisfied by same-engine execution order
                        kept.append(w)
                    while len(kept) > cap:
                        ncarry = min(2, len(kept) - cap + 1)
                        carry, kept = kept[:ncarry], kept[ncarry:]
                        ev = mybir.InstEventSemaphore(
                            name=f"EVW-{ev_id}", ins=[], outs=[]
                        )
                        ev_id += 1
                        ev.engine = eng
                        ev.sync_info = mybir.SyncInfo(on_wait=carry, on_update=[])
                        new_insts.append(ev)
                    inst.sync_info = mybir.SyncInfo(
                        on_wait=kept, on_update=si.on_update
                    )
                si = inst.sync_info
                if si is not None:
                    for u in si.on_update:
                        if u.update_mode == "sem-inc":
                            key = (inst.engine, u.ant_name)
                            sem_incs[key] = sem_incs.get(key, 0) + u.update_value
                new_insts.append(inst)
            if len(new_insts) != len(insts):
                insts[:] = new_insts


def _build_nc():
    nc = bass.Bass("TRN2", debug=False, num_devices=NCORES)

    # f_t[p, t*128 + b] = F[b, t*128 + p]  (bf16 transposed F tiles)
    f_t_in = nc.dram_tensor("f_t_in", [P, N], BF16, kind="ExternalInput").ap()
    # tabs16[p, 0:G] = iota_g; [G:G+2] = (p, p+128); [G+2+r*NT+t] =
    # labels[r, t*128+p] -- one u16 table tensor, one DMA
    tabs16 = nc.dram_tensor(
        "tabs16", [P, G + 2 + R * NT], U16, kind="ExternalInput"
    ).ap()
    # m28[r, g, h, j] = (labels[r, j] == h*128 + g), fp8e4 (host-built
    # one-hot gather masks, DoubleRow-packed: h is the k-tile axis)
    m28_in = nc.dram_tensor("m28", [R, P, 2, N], FP8, kind="ExternalInput").ap()
    # channel-major output: out2[b, r, j] = seg_r[b, lab_r[j]]
    out2 = nc.dram_tensor("out2", [BL, R, N], BF16, kind="ExternalOutput").ap()

    with ExitStack() as ctx:
        tc = ctx.enter_context(tile.TileContext(nc))

        const = ctx.enter_context(tc.tile_pool(name="const", bufs=1))
        m1p = ctx.enter_context(tc.tile_pool(name="m1p", bufs=52))
        m1pp = ctx.enter_context(tc.tile_pool(name="m1pp", bufs=22))
        segp = ctx.enter_context(tc.tile_pool(name="segp", bufs=3))
        chp = ctx.enter_context(tc.tile_pool(name="chp", bufs=4))
        ps_seg = ctx.enter_context(tc.tile_pool(name="ps_seg", bufs=2, space="PSUM"))

        # ---- input DMAs. sync (SP) queue: tables, then f_t (compute-
        # critical, in quarters so stage-1 starts early), then the 6 per-rank
        # u16 label broadcasts (rank r's is needed only by its m2 build). ----
        t16_sb = const.tile([P, G + 2 + R * NT], U16)
        nc.sync.dma_start(t16_sb[:], tabs16)
        f_t = const.tile([P, N], BF16)
        j0 = 0
        for cw in (256, 384, 512, 640, 768, 896, 640):
            nc.sync.dma_start(f_t[:, j0 : j0 + cw], f_t_in[:, j0 : j0 + cw])
            j0 += cw
        # masks for the first three ranks load up-front; the rest are issued
        # inside the rank loop so their transfers fill the DMA idle during
        # late stage-1 phases instead of front-running the output stream
        m2_sb = const.tile([P, R, 2, N], FP8)
        for r in range(3):
            nc.sync.dma_start(m2_sb[:, r], m28_in[r])

        iota_g_sb = t16_sb[:, 0:G]
        # is_equal scalars must be f32: convert the iota_p/labT columns once
        scal_sb = const.tile([P, 2 + R * NT], F32)
        nc.scalar.copy(scal_sb[:], t16_sb[:, G:])
        labT_sb = scal_sb[:, 2:]

        # ---- prewarm: absorb const-DMA semaphores into the DVE/Pool clocks
        # (hot-loop ops may carry at most one sync wait), and keep PE busy on
        # junk matmuls until f_t arrives so its p-state ramps. ----
        warm = const.tile([P, 4], BF16)
        wjunk = const.tile([P, P], BF16)
        nc.vector.memset(wjunk[:], 0.0)
        nc.vector.tensor_copy(warm[:, 0:1], t16_sb[:, 0:1])
        nc.gpsimd.tensor_copy(warm[:, 2:3], t16_sb[:, 1:2])
        with tc.tile_pool(name="ps_warm", bufs=1, space="PSUM") as ps_warm:
            wps = ps_warm.tile([P, P], F32)
            # p-state warm-up on UNINITIALIZED tiles: no input dependency, so
            # PE is busy from ~0.6us (the product is never read)
            for _ in range(25):
                nc.tensor.matmul(wps[:], wjunk[:], wjunk[:], start=True, stop=True)
        # quad po pool enters after the warm psum bank is released
        ps_o = ctx.enter_context(tc.tile_pool(name="ps_o", bufs=6, space="PSUM"))

        # ---- per-rank emitters ----
        seg_ps = {}
        seg_t = {}

        def emit_m1(r):
            masks = []
            pool_t = POOL_T_RAMP if r < 2 else POOL_T
            for t in range(NT):
                col = r * NT + t
                if t in pool_t:
                    mt = m1pp.tile([P, G], BF16, tag="m1p", name=f"m1p{r}_{t}")
                    eng = nc.gpsimd
                else:
                    mt = m1p.tile([P, G], BF16, tag="m1", name=f"m1_{r}_{t}")
                    eng = nc.vector
                eng.tensor_scalar(
                    mt[:],
                    iota_g_sb[:],
                    labT_sb[:, col : col + 1],
                    None,
                    op0=mybir.AluOpType.is_equal,
                )
                masks.append(mt)
            return masks

        def emit_s1(r, masks):
            # gh outer / t inner: interleaved accumulation groups miscompute
            # on HW (probe-verified), so groups must stay contiguous
            t_ = ps_seg.tile([P, 2, P], F32, tag="segps", name=f"segps{r}")
            seg_ps[r] = t_
            for gh in range(2):
                for t in range(NT):
                    nc.tensor.matmul(
                        t_[:, gh, :],
                        masks[t][:, ts(gh, P)],
                        f_t[:, ts(t, P)],
                        start=(t == 0),
                        stop=(t == NT - 1),
                    )

        def emit_seg_copy(r):
            # seg -> fp8 hi (seg8) + fp8 residual (res8); seg8 + res8
            # reconstructs seg to ~2^-8 relative. Three-op chain: the
            # subtract reads the psum directly and emits fp8.
            s8 = segp.tile([P, 2, P], FP8, tag="s8", name=f"s8_{r}")
            nc.scalar.copy(s8[:], seg_ps[r][:])
            s8b = segp.tile([P, 2, P], BF16, tag="s8b", name=f"s8b{r}")
            nc.vector.tensor_copy(s8b[:], s8[:])
            r8 = segp.tile([P, 2, P], FP8, tag="r8", name=f"r8_{r}")
            nc.vector.tensor_tensor(
                r8[:], seg_ps[r][:], s8b[:], op=mybir.AluOpType.subtract
            )
            seg_t[r] = (s8, r8)

        ch_of = {}
        W2 = 256  # DoubleRow strip width (rhs free = 2*W2 = PE moving max)

        def emit_s2(r, p_lo=0, p_hi=8):
            # stage-2: per 256-strip, two fp8 DoubleRow matmuls (seg8 then
            # res8) contract all 256 groups at 0.5 cyc/row. Strips pair up in
            # one psum tile so each po->channel copy moves 512 columns.
            if r in ch_of:
                ch = ch_of[r]
            else:
                ch = chp.tile([P, N], BF16, tag="ch", name=f"ch{r}")
                ch_of[r] = ch
            s8, r8 = seg_t[r]
            for p_ in range(p_lo, p_hi):
                po = ps_o.tile([P, 2, W2], F32, tag="po", name=f"po{r}_{p_}")
                for half in range(2):
                    s = 2 * p_ + half
                    for op8 in (s8, r8):
                        nc.tensor.matmul(
                            po[:, half, :],
                            op8[:],
                            m2_sb[:, r, :, ts(s, W2)],
                            start=(op8 is s8),
                            stop=(op8 is r8),
                            perf_mode=mybir.MatmulPerfMode.DoubleRow,
                        )
                # GPSIMD cannot read PSUM (walrus birverifier), so copies
                # split across ACT and DVE only
                if r >= R - 3:
                    eng = nc.vector if p_ % 2 == 1 else nc.scalar
                else:
                    eng = nc.vector if p_ % 4 == 2 else nc.scalar
                if eng is nc.scalar:
                    nc.scalar.copy(ch[:, ts(p_, 2 * W2)], po[:])
                else:
                    eng.tensor_copy(ch[:, ts(p_, 2 * W2)], po[:])
                # stream in quarters (2 pairs each): short final drain,
                # cheap per-DMA HWDGE hold (625ns, exclusive)
                if p_ % 2 == 1:
                    q = p_ // 2
                    nc.sync.dma_start(
                        out2[:, r, ts(q, N // 4)], ch[:, ts(q, N // 4)]
                    )

        # ---- software-pipelined emission: PE order is
        # s1(0) s1(1) s2(0) s1(2) s2(1) ... s1(5) s2(4) s2(5), so PE always
        # has a stage-1 queued while stage-2 deps (m2, seg copies) settle.
        # m2(r+1) is emitted during s2(r)'s window so DVE never idles on a
        # label DMA and each m2 is ready one full window early. ----
        masks0 = emit_m1(0)
        emit_s1(0, masks0)
        emit_seg_copy(0)
        masks1 = emit_m1(1)
        emit_s1(1, masks1)
        emit_seg_copy(1)
        for r in range(3):
            if r + 3 < R:
                nc.sync.dma_start(m2_sb[:, r + 3], m28_in[r + 3])
            emit_s2(r)
            masks = emit_m1(r + 2)
            emit_s1(r + 2, masks)
            emit_seg_copy(r + 2)
        # tail: s2(4) runs BEFORE s1(5), so rank 4's channel streams while
        # PE does rank 5's stage-1 (a window where DMA would otherwise
        # idle), and only rank 5's channel remains after the last matmul
        emit_s2(3)
        masks5 = emit_m1(5)
        emit_s2(4, 0, 6)
        emit_s1(5, masks5)
        emit_seg_copy(5)
        emit_s2(4, 6, 8)
        emit_s2(5)

    _legalize_waits(nc)
    return nc


def _host_tables(labels):
    # tabs16 = [iota_g | iota_p, iota_p+128 | labT] as u16
    iota_g = np.tile(np.arange(G, dtype=np.uint16), (P, 1))
    iota_p = (
        np.arange(P, dtype=np.uint16)[:, None]
        + np.uint16(128) * np.arange(2, dtype=np.uint16)[None, :]
    )
    labT = (
        np.transpose(labels.reshape(R, NT, P), (2, 0, 1))
        .reshape(P, R * NT)
        .astype(np.uint16)
    )
    return np.ascontiguousarray(
        np.concatenate([iota_g, iota_p, labT], axis=1).astype(np.uint16)
    )


def kernel(F_genus: np.ndarray, labels: np.ndarray) -> np.ndarray:
    F_genus = np.ascontiguousarray(F_genus, dtype=np.float32)
    labels = np.ascontiguousarray(labels, dtype=np.int32)
    assert F_genus.shape == (B, N) and labels.shape == (R, N)

    tabs16 = _host_tables(labels)
    # m28[r, g, h, j] = (labels[r, j] == h*128 + g) in fp8e4 (exact 0/1)
    gvals = (
        np.arange(P, dtype=np.int32)[None, :, None, None]
        + 128 * np.arange(2, dtype=np.int32)[None, None, :, None]
    )
    m28 = np.ascontiguousarray(
        (labels[:, None, None, :] == gvals).astype(ml_dtypes.float8_e4m3fn)
    )

    in_maps = []
    for c in range(NCORES):
        Fc = F_genus[c * BL : (c + 1) * BL]  # [BL, N]
        # f_t[p, t*128 + b] = Fc[b, t*128 + p]
        f_t = np.ascontiguousarray(
            Fc.reshape(BL, NT, P).transpose(2, 1, 0).reshape(P, N)
        ).astype(ml_dtypes.bfloat16)
        in_maps.append(
            {
                "f_t_in": f_t,
                "tabs16": tabs16,
                "m28": m28,
            }
        )

    # The first execution of a freshly compiled NEFF occasionally hits a
    # transient NRT_EXEC_UNIT_UNRECOVERABLE; a rebuild + retry recovers.
    last_err = None
    for attempt in range(3):
        try:
            if "nc" not in _cache:
                _cache["nc"] = _build_nc()
            res = run_bass_kernel_spmd(
                _cache["nc"], in_maps, core_ids=list(range(NCORES))
            )
            out = np.empty((B, N, R + 1), dtype=np.float32)
            for c in range(NCORES):
                # out2 is [BL, R, N] bf16, channel-major -> transpose
                out[c * BL : (c + 1) * BL, :, :R] = (
                    res.results[c]["out2"].astype(np.float32).transpose(0, 2, 1)
                )
            out[:, :, R] = F_genus  # identity channel, exact
            return out
        except Exception as e:  # noqa: BLE001
            last_err = e
            _cache.pop("nc", None)
            import time as _time

            _time.sleep(3.0)
    raise last_err
